# revision 29
# baseline (speedup 1.0000x reference)
"""Trainium2 Bass kernel for nn_BilinearDecoder: bilinear logits + diag mask +
bernoulli sampling + entropy, data-parallel over batch on 8 NeuronCores.

Math per batch b (reference):
    logits = E_b @ W @ E_b^T + l            [L, L]
    masked = logits - 1e8 * eye(L)
    p      = sigmoid(masked)
    samples= bernoulli(key(42), p)          == (u < p) == (masked > logit(u))
    entropy= p*softplus(-masked) + (1-p)*softplus(masked)
           = softplus(-a) - silu(-a),  a = |masked|   (even fn; all terms
                                                       bounded, no
                                                       cancellation)

Device strategy (per core, 4 batches of [L, L] output):
    f32r matmuls (full PE rate, ~1e-4 rel err -> ~2k sample flips, well
    under the 2e-2 norm gate; masked is stored fp16 anyway).
    Two ACT table sets are needed (exp/ln for softplus, silu); work runs in
    two sweeps per 2-batch group so each set loads once per group, with
    scheduler gates (tile_wait_until) keeping the Silu block out of the
    Exp/Ln stream.  Group g's Silu sweep overlaps group g+1's matmul/DVE
    sweep.  Per 512-row block (2 PSUM pairs):
      DVE   masked_f16 = max(ps_x + l, -65300)   (store tile; finite diag)
      DVE   a_f16      = |masked|  (fp16 sign-bit clear)  -> a_grp (SBUF)
      DVE   samples    = (masked > t)  in place over the t tile, f16
      ACT   e = Exp(-a) into lg_grp;  lg = Ln(e+1) in place  [sweep A]
      ACT   v = Silu(-a)                                     [sweep B]
      POOL/DVE ent = lg - v, in place over v  (POOL overlaps the next
               group's sweep A; DVE for the last group to avoid a slow tail)
    t = logit(u) fp16 precomputed host-side (fixed key(42) constant).
    HBM traffic per core: 34 MB (t 8 + enc 2 + masked 8 + samples 8 + ent 8).
    Host: outputs upcast to f32; masked diagonal overwritten with -1e8
    (true value is -1e8 + O(60); rel err <= 6e-7 per element).
"""
import sys
import json

sys.path.insert(0, '/opt/trn_rl_repo')

import numpy as np
import concourse.bass as bass
import concourse.tile as tile
from concourse import mybir
from concourse.masks import make_identity
from concourse.bass_utils import run_bass_kernel_spmd

# Problem shapes (hardcoded per contest rules)
B, L, H = 32, 1024, 128
N_CORES = 8
BPC = B // N_CORES           # batches per core
NCHUNK = L // 128            # 128-row chunks per batch
NEG_BIG = 1.0e8

F32 = mybir.dt.float32
F32R = mybir.dt.float32r
F16 = mybir.dt.float16
U16 = mybir.dt.uint16
GROUPB = 2                   # batches per ACT-table sweep group
GCHUNK = GROUPB * NCHUNK     # chunks per group


def _split_waits_bir(d, limit=1):
    """This container's walrus accepts only `limit` sync-wait commands per
    instruction; Tile's kernel-tail drain carries several.  Move extras onto
    preceding Drain carriers on the same engine (order-preserving, safe)."""
    n = 0
    for fn in d['functions']:
        for bb in fn['blocks']:
            new_ins = []
            for ins in bb.get('instructions', []):
                si = ins.get('sync_info') or {}
                ow = si.get('on_wait') or []
                if len(ow) > limit:
                    extra = ow[:-limit]
                    si['on_wait'] = ow[-limit:]
                    for w in extra:
                        n += 1
                        new_ins.append({
                            "debug": ins.get("debug", 0),
                            "engine": ins["engine"],
                            "ins": [], "outs": [],
                            "is_reset_sema": False,
                            "name": f"{ins['name']}-wsplit{n}",
                            "opcode": "NoOp",
                            "sync_info": {"on_update": [], "on_wait": [w]},
                        })
                new_ins.append(ins)
            bb['instructions'] = new_ins
    return n


class PatchedBass(bass.Bass):
    def to_json_bytes(self):
        d = json.loads(super().to_json_bytes())
        _split_waits_bir(d)
        return json.dumps(d).encode()


def _build_nc():
    nc = PatchedBass("TRN2")

    enc = nc.dram_tensor("enc", [BPC, H, L], F32R, kind="ExternalInput")
    w_in = nc.dram_tensor("w_in", [H, H], F32R, kind="ExternalInput")
    lbias = nc.dram_tensor("lbias", [1], F32, kind="ExternalInput")
    thr = nc.dram_tensor("thr", [BPC, NCHUNK // 4, 128, 4, L], F16,
                         kind="ExternalInput")

    samples_o = nc.dram_tensor("samples_o", [L, BPC, L], F16, kind="ExternalOutput")
    masked_o = nc.dram_tensor("masked_o", [L, BPC, L], F16, kind="ExternalOutput")
    entropy_o = nc.dram_tensor("entropy_o", [L, BPC, L], F16, kind="ExternalOutput")

    with tile.TileContext(nc) as tc:
        with (
            tc.tile_pool(name="consts", bufs=1) as consts,
            tc.tile_pool(name="x_ps", bufs=2, space="PSUM") as x_ps,
            tc.tile_pool(name="etbuf", bufs=2) as etbuf,
            tc.tile_pool(name="kbuf", bufs=2) as kbuf,
            tc.tile_pool(name="tpool", bufs=2) as tpool,
            tc.tile_pool(name="mpool", bufs=3) as mpool,
            tc.tile_pool(name="vpool", bufs=2) as vpool,
            tc.tile_pool(name="abuf3", bufs=1) as abuf3,
            tc.tile_pool(name="rbuf3", bufs=1) as rbuf3,
            tc.tile_pool(name="abuf1", bufs=1) as abuf1,
            tc.tile_pool(name="rbuf1", bufs=1) as rbuf1,
        ):
            # ---- constants ----
            ident = consts.tile([128, 128], F32)
            make_identity(nc, ident[:])
            neg_eye = consts.tile([128, 128], F32)
            nc.vector.tensor_scalar_mul(neg_eye[:], ident[:], -NEG_BIG)
            # l broadcast to [128, 1] (per-partition bias operand)
            l_bc = consts.tile([128, 1], F32)
            l_bcast_ap = bass.AP(tensor=lbias, offset=0, ap=[[0, 128], [1, 1]])
            nc.gpsimd.dma_start(out=l_bc[:], in_=l_bcast_ap)

            # ---- W^T comes pre-transposed from the host shard copy ----
            wt = consts.tile([128, 128], F32R)
            nc.sync.dma_start(out=wt[:], in_=w_in[:, :])

            groups = [(0, [0, 1, 2]), (1, [3])]
            for g, bs in groups:
                # a/lg survive the whole group (sweep A writes, sweep B reads)
                ab = abuf3 if g == 0 else abuf1
                rb = rbuf3 if g == 0 else rbuf1
                a_grp = ab.tile([128, len(bs) * NCHUNK, L], F16)
                lg_grp = rb.tile([128, len(bs) * NCHUNK, L], F16)

                # ================= sweep A (natural_log_exp set) ============
                tc.tile_set_cur_wait(0.0)
                act_gate = 0.0 if g == 0 else 0.060
                for bi, b in enumerate(bs):
                    # ---- E_b^T comes pre-transposed from the host shard
                    # copy: one contiguous load, no PE transposes ----
                    et = etbuf.tile([128, L], F32R)
                    nc.sync.dma_start(out=et[:], in_=enc[b])
                    ps_prep = x_ps.tile([128, 2, 1024], F32, tag="x")
                    for half in range(2):
                        sl = slice(half * 512, (half + 1) * 512)
                        nc.tensor.matmul(ps_prep[:, 1, sl], wt[:],
                                         et[:, sl], start=True, stop=True)
                    kb = kbuf.tile([128, L], F32R)
                    nc.vector.tensor_copy(kb[:], ps_prep[:, 1, :])

                    for c4 in range(2):
                        t4 = tpool.tile([128, 4, L], F16)
                        nc.sync.dma_start(out=t4[:], in_=thr[b, c4])
                        rows4 = slice(c4 * 512, (c4 + 1) * 512)
                        masked_t = mpool.tile([128, 4, L], F16)

                        for i2 in range(2):
                            # ---- x = E W E^T - 1e8*eye  (PSUM, f32r) ----
                            ps_x = x_ps.tile([128, 2, 1024], F32, tag="x")
                            for i in range(2):
                                c = 4 * c4 + 2 * i2 + i
                                rows = slice(c * 128, (c + 1) * 128)
                                for half in range(2):
                                    sl = slice(half * 512, (half + 1) * 512)
                                    diag_here = (c * 128 >= sl.start) and (c * 128 < sl.stop)
                                    nc.tensor.matmul(
                                        ps_x[:, i, sl], et[:, rows], kb[:, sl],
                                        start=True, stop=not diag_here,
                                    )
                                nc.tensor.matmul(
                                    ps_x[:, i, rows], neg_eye[:], ident[:],
                                    start=False, stop=True,
                                )

                            # ---- masked = max(x + l, -65300)  (f16) ----
                            nc.vector.tensor_scalar(
                                masked_t[:, 2 * i2:2 * i2 + 2, :], ps_x[:],
                                l_bc[:, 0:1], -65300.0,
                                op0=mybir.AluOpType.add, op1=mybir.AluOpType.max,
                            )

                        asl = slice(bi * NCHUNK + 4 * c4, bi * NCHUNK + 4 * c4 + 4)
                        # ---- a = |masked| via fp16 sign-bit clear ----
                        nc.vector.tensor_scalar(
                            a_grp[:, asl, :].bitcast(U16),
                            masked_t[:].bitcast(U16), 0x7FFF, None,
                            op0=mybir.AluOpType.bitwise_and,
                        )

                        # ---- samples = (masked > t), in place over t4 ----
                        nc.vector.tensor_tensor(
                            t4[:], masked_t[:], t4[:],
                            op=mybir.AluOpType.is_gt,
                        )

                        # ---- lg = softplus(-a) = ln(exp(-a) + 1); the Exp
                        # writes into the lg slot, Ln runs in place.  Only
                        # these ACT ops carry the scheduling gate (keeps the
                        # Exp/Ln stream clear of the other table set without
                        # delaying this group's DVE/PE work) ----
                        with tc.tile_wait_until(act_gate):
                            nc.scalar.activation(
                                lg_grp[:, asl, :], a_grp[:, asl, :],
                                mybir.ActivationFunctionType.Exp, scale=-1.0,
                            )
                            nc.scalar.activation(
                                lg_grp[:, asl, :], lg_grp[:, asl, :],
                                mybir.ActivationFunctionType.Ln, bias=1.0,
                            )

                        # ---- stores (1 MB each) ----
                        nc.sync.dma_start(
                            out=masked_o[rows4, b, :].rearrange("(t p) l -> p t l", p=128),
                            in_=masked_t[:],
                        )
                        nc.sync.dma_start(
                            out=samples_o[rows4, b, :].rearrange("(t p) l -> p t l", p=128),
                            in_=t4[:],
                        )

                # ================= sweep B (silu table set) =================
                tc.tile_set_cur_wait(0.060 if g == 0 else 0.105)
                for bi, b in enumerate(bs):
                    for c4 in range(2):
                        rows4 = slice(c4 * 512, (c4 + 1) * 512)
                        asl = slice(bi * NCHUNK + 4 * c4, bi * NCHUNK + 4 * c4 + 4)
                        v_t = vpool.tile([128, 4, L], F16)
                        nc.scalar.activation(
                            v_t[:], a_grp[:, asl, :],
                            mybir.ActivationFunctionType.Silu, scale=-1.0,
                        )
                        # ent = lg - v, in place over v.  POOL runs ~2x slow
                        # under SBUF-port contention, so alternate with DVE in
                        # the big group and keep the last group off POOL
                        # entirely (no slow tail).
                        if g == 1:
                            nc.vector.tensor_sub(v_t[:], lg_grp[:, asl, :], v_t[:])
                        else:
                            nc.gpsimd.tensor_sub(v_t[:], lg_grp[:, asl, :], v_t[:])
                        nc.sync.dma_start(
                            out=entropy_o[rows4, b, :].rearrange("(t p) l -> p t l", p=128),
                            in_=v_t[:],
                        )

    return nc


_NC = None
_THR = None


def _get_nc():
    global _NC
    if _NC is None:
        _NC = _build_nc()
    return _NC


def _get_thr():
    """t = logit(u) with u = the exact uniforms jax.random.bernoulli(key(42))
    draws inside the reference.  Input-independent => precomputed constant.
    fp16 threshold rounding flips ~300 of 33.5M samples (norm gate allows
    ~6700).  u == 0 gives t = -inf; clamp to -65000 so the fp16 diag
    (clamped to ~-65300) still compares below it."""
    global _THR
    if _THR is None:
        import jax
        cpu = jax.devices("cpu")[0]
        with jax.default_device(cpu):
            u = np.asarray(
                jax.random.uniform(
                    jax.random.key(42), (L, B, L), dtype=np.float32
                )
            )
        u64 = u.astype(np.float64)
        with np.errstate(divide="ignore"):
            t = np.log(u64) - np.log1p(-u64)
        t = np.clip(t, -65000.0, 65000.0)
        _THR = t.astype(np.float16)
    return _THR


def _shard_inputs(encoder_output, W, l):
    """Build the per-core input maps (also used by test.py)."""
    encoder_output = np.ascontiguousarray(encoder_output, dtype=np.float32)
    W = np.ascontiguousarray(W, dtype=np.float32)
    l = np.ascontiguousarray(l, dtype=np.float32)
    thr = _get_thr()
    in_maps = []
    for i in range(N_CORES):
        bs = slice(i * BPC, (i + 1) * BPC)
        shard = thr[:, bs, :]
        # [L, BPC, L] -> [BPC, L/512, 128, 4, L]: row l = s*512 + t*128 + p
        tiled = np.ascontiguousarray(
            shard.reshape(NCHUNK // 4, 4, 128, BPC, L)
            .transpose(3, 0, 2, 1, 4)
        )
        in_maps.append({
            "enc": np.ascontiguousarray(encoder_output[bs].transpose(0, 2, 1)),
            "w_in": np.ascontiguousarray(W.T),
            "lbias": l,
            "thr": tiled,
        })
    return in_maps


def _unshard(results):
    samples = np.concatenate(
        [np.asarray(r["samples_o"]).astype(np.float32) for r in results], axis=1)
    masked = np.concatenate(
        [np.asarray(r["masked_o"]).astype(np.float32) for r in results], axis=1)
    entropy = np.concatenate(
        [np.asarray(r["entropy_o"]).astype(np.float32) for r in results], axis=1)
    # fp16 clamps the diagonal (-1e8 -> -65300); true value is
    # -1e8 + logits_ii + l = -1e8 * (1 + O(6e-7)): write the constant.
    ar = np.arange(L)
    masked[ar, :, ar] = -NEG_BIG
    return samples, masked, entropy


def kernel(encoder_output, W, l):
    in_maps = _shard_inputs(encoder_output, W, l)
    nc = _get_nc()
    res = run_bass_kernel_spmd(nc, in_maps, core_ids=list(range(N_CORES)))
    return _unshard(res.results)


# revision 30
# speedup vs baseline: 1.1770x; 1.1770x over previous
"""Trainium2 Bass kernel for nn_BilinearDecoder: bilinear logits + diag mask +
bernoulli sampling + entropy, data-parallel over batch on 8 NeuronCores.

Math per batch b (reference):
    logits = E_b @ W @ E_b^T + l            [L, L]
    masked = logits - 1e8 * eye(L)
    p      = sigmoid(masked)
    samples= bernoulli(key(42), p)          == (u < p) == (masked > logit(u))
    entropy= p*softplus(-masked) + (1-p)*softplus(masked)
           = softplus(-a) - silu(-a),  a = |masked|   (even fn; all terms
                                                       bounded, no
                                                       cancellation)

Device strategy (per core, 4 batches of [L, L] output):
    f32r matmuls (full PE rate, ~1e-4 rel err -> ~2k sample flips, well
    under the 2e-2 norm gate; masked is stored fp16 anyway).
    Two ACT table sets are needed (exp/ln for softplus, silu); work runs in
    two sweeps per 2-batch group so each set loads once per group, with
    scheduler gates (tile_wait_until) keeping the Silu block out of the
    Exp/Ln stream.  Group g's Silu sweep overlaps group g+1's matmul/DVE
    sweep.  Per 512-row block (2 PSUM pairs):
      DVE   masked_f16 = max(ps_x + l, -65300)   (store tile; finite diag)
      DVE   a_f16      = |masked|  (fp16 sign-bit clear)  -> a_grp (SBUF)
      DVE   samples    = (masked > t)  in place over the t tile, f16
      ACT   e = Exp(-a) into lg_grp;  lg = Ln(e+1) in place  [sweep A]
      ACT   v = Silu(-a)                                     [sweep B]
      POOL/DVE ent = lg - v, in place over v  (POOL overlaps the next
               group's sweep A; DVE for the last group to avoid a slow tail)
    t = logit(u) fp16 precomputed host-side (fixed key(42) constant).
    HBM traffic per core: 34 MB (t 8 + enc 2 + masked 8 + samples 8 + ent 8).
    Host: outputs upcast to f32; masked diagonal overwritten with -1e8
    (true value is -1e8 + O(60); rel err <= 6e-7 per element).
"""
import sys
import json

sys.path.insert(0, '/opt/trn_rl_repo')

import numpy as np
import concourse.bass as bass
import concourse.tile as tile
from concourse import mybir
from concourse.masks import make_identity
from concourse.bass_utils import run_bass_kernel_spmd

# Problem shapes (hardcoded per contest rules)
B, L, H = 32, 1024, 128
N_CORES = 8
BPC = B // N_CORES           # batches per core
NCHUNK = L // 128            # 128-row chunks per batch
NEG_BIG = 1.0e8

F32 = mybir.dt.float32
F32R = mybir.dt.float32r
F16 = mybir.dt.float16
U16 = mybir.dt.uint16
GROUPB = 2                   # batches per ACT-table sweep group
GCHUNK = GROUPB * NCHUNK     # chunks per group


def _split_waits_bir(d, limit=1):
    """This container's walrus accepts only `limit` sync-wait commands per
    instruction; Tile's kernel-tail drain carries several.  Move extras onto
    preceding Drain carriers on the same engine (order-preserving, safe)."""
    n = 0
    for fn in d['functions']:
        for bb in fn['blocks']:
            new_ins = []
            for ins in bb.get('instructions', []):
                si = ins.get('sync_info') or {}
                ow = si.get('on_wait') or []
                if len(ow) > limit:
                    extra = ow[:-limit]
                    si['on_wait'] = ow[-limit:]
                    for w in extra:
                        n += 1
                        new_ins.append({
                            "debug": ins.get("debug", 0),
                            "engine": ins["engine"],
                            "ins": [], "outs": [],
                            "is_reset_sema": False,
                            "name": f"{ins['name']}-wsplit{n}",
                            "opcode": "NoOp",
                            "sync_info": {"on_update": [], "on_wait": [w]},
                        })
                new_ins.append(ins)
            bb['instructions'] = new_ins
    return n


class PatchedBass(bass.Bass):
    def to_json_bytes(self):
        d = json.loads(super().to_json_bytes())
        _split_waits_bir(d)
        return json.dumps(d).encode()


def _build_nc():
    nc = PatchedBass("TRN2")

    enc = nc.dram_tensor("enc", [BPC, H, L], F32R, kind="ExternalInput")
    w_in = nc.dram_tensor("w_in", [H, H], F32R, kind="ExternalInput")
    lbias = nc.dram_tensor("lbias", [1], F32, kind="ExternalInput")
    thr = nc.dram_tensor("thr", [BPC, NCHUNK // 4, 128, 4, L], F16,
                         kind="ExternalInput")

    samples_o = nc.dram_tensor("samples_o", [L, BPC, L], F16, kind="ExternalOutput")
    masked_o = nc.dram_tensor("masked_o", [L, BPC, L], F16, kind="ExternalOutput")
    entropy_o = nc.dram_tensor("entropy_o", [L, BPC, L], F16, kind="ExternalOutput")

    with tile.TileContext(nc) as tc:
        with (
            tc.tile_pool(name="consts", bufs=1) as consts,
            tc.tile_pool(name="x_ps", bufs=2, space="PSUM") as x_ps,
            tc.tile_pool(name="etbuf", bufs=2) as etbuf,
            tc.tile_pool(name="kbuf", bufs=2) as kbuf,
            tc.tile_pool(name="tpool", bufs=2) as tpool,
            tc.tile_pool(name="mpool", bufs=2) as mpool,
            tc.tile_pool(name="vpool", bufs=3) as vpool,
            tc.tile_pool(name="abuf3", bufs=1) as abuf3,
            tc.tile_pool(name="rbuf3", bufs=1) as rbuf3,
            tc.tile_pool(name="abuf1", bufs=1) as abuf1,
            tc.tile_pool(name="rbuf1", bufs=1) as rbuf1,
        ):
            # ---- constants ----
            ident = consts.tile([128, 128], F32)
            make_identity(nc, ident[:])
            neg_eye = consts.tile([128, 128], F32)
            nc.vector.tensor_scalar_mul(neg_eye[:], ident[:], -NEG_BIG)
            # l broadcast to [128, 1] (per-partition bias operand)
            l_bc = consts.tile([128, 1], F32)
            l_bcast_ap = bass.AP(tensor=lbias, offset=0, ap=[[0, 128], [1, 1]])
            nc.gpsimd.dma_start(out=l_bc[:], in_=l_bcast_ap)

            # ---- W^T comes pre-transposed from the host shard copy ----
            wt = consts.tile([128, 128], F32R)
            nc.sync.dma_start(out=wt[:], in_=w_in[:, :])

            groups = [(0, [0, 1, 2]), (1, [3])]
            for g, bs in groups:
                # a/lg survive the whole group (sweep A writes, sweep B reads)
                ab = abuf3 if g == 0 else abuf1
                rb = rbuf3 if g == 0 else rbuf1
                a_grp = ab.tile([128, len(bs) * NCHUNK, L], F16)
                lg_grp = rb.tile([128, len(bs) * NCHUNK, L], F16)

                # ================= sweep A (natural_log_exp set) ============
                tc.tile_set_cur_wait(0.0)
                act_gate = 0.0 if g == 0 else 0.060
                for bi, b in enumerate(bs):
                    # ---- E_b^T comes pre-transposed from the host shard
                    # copy: one contiguous load, no PE transposes ----
                    et = etbuf.tile([128, L], F32R)
                    nc.sync.dma_start(out=et[:], in_=enc[b])
                    ps_prep = x_ps.tile([128, 2, 1024], F32, tag="x")
                    for half in range(2):
                        sl = slice(half * 512, (half + 1) * 512)
                        nc.tensor.matmul(ps_prep[:, 1, sl], wt[:],
                                         et[:, sl], start=True, stop=True)
                    kb = kbuf.tile([128, L], F32R)
                    nc.vector.tensor_copy(kb[:], ps_prep[:, 1, :])

                    for c4 in range(2):
                        t4 = tpool.tile([128, 4, L], F16)
                        nc.sync.dma_start(out=t4[:], in_=thr[b, c4])
                        rows4 = slice(c4 * 512, (c4 + 1) * 512)
                        masked_t = mpool.tile([128, 4, L], F16)

                        for i2 in range(2):
                            # ---- x = E W E^T - 1e8*eye  (PSUM, f32r) ----
                            ps_x = x_ps.tile([128, 2, 1024], F32, tag="x")
                            for i in range(2):
                                c = 4 * c4 + 2 * i2 + i
                                rows = slice(c * 128, (c + 1) * 128)
                                for half in range(2):
                                    sl = slice(half * 512, (half + 1) * 512)
                                    diag_here = (c * 128 >= sl.start) and (c * 128 < sl.stop)
                                    nc.tensor.matmul(
                                        ps_x[:, i, sl], et[:, rows], kb[:, sl],
                                        start=True, stop=not diag_here,
                                    )
                                nc.tensor.matmul(
                                    ps_x[:, i, rows], neg_eye[:], ident[:],
                                    start=False, stop=True,
                                )

                            # ---- masked = max(x + l, -65300)  (f16) ----
                            nc.vector.tensor_scalar(
                                masked_t[:, 2 * i2:2 * i2 + 2, :], ps_x[:],
                                l_bc[:, 0:1], -65300.0,
                                op0=mybir.AluOpType.add, op1=mybir.AluOpType.max,
                            )

                        asl = slice(bi * NCHUNK + 4 * c4, bi * NCHUNK + 4 * c4 + 4)
                        # ---- a = |masked| via fp16 sign-bit clear ----
                        nc.vector.tensor_scalar(
                            a_grp[:, asl, :].bitcast(U16),
                            masked_t[:].bitcast(U16), 0x7FFF, None,
                            op0=mybir.AluOpType.bitwise_and,
                        )

                        # ---- samples = (masked > t), in place over t4 ----
                        nc.vector.tensor_tensor(
                            t4[:], masked_t[:], t4[:],
                            op=mybir.AluOpType.is_gt,
                        )

                        # ---- lg = softplus(-a) = ln(exp(-a) + 1); the Exp
                        # writes into the lg slot, Ln runs in place.  Only
                        # these ACT ops carry the scheduling gate (keeps the
                        # Exp/Ln stream clear of the other table set without
                        # delaying this group's DVE/PE work) ----
                        with tc.tile_wait_until(act_gate):
                            nc.scalar.activation(
                                lg_grp[:, asl, :], a_grp[:, asl, :],
                                mybir.ActivationFunctionType.Exp, scale=-1.0,
                            )
                            nc.scalar.activation(
                                lg_grp[:, asl, :], lg_grp[:, asl, :],
                                mybir.ActivationFunctionType.Ln, bias=1.0,
                            )

                        # ---- stores (1 MB each) ----
                        nc.sync.dma_start(
                            out=masked_o[rows4, b, :].rearrange("(t p) l -> p t l", p=128),
                            in_=masked_t[:],
                        )
                        nc.sync.dma_start(
                            out=samples_o[rows4, b, :].rearrange("(t p) l -> p t l", p=128),
                            in_=t4[:],
                        )

                # ================= sweep B (silu table set) =================
                tc.tile_set_cur_wait(0.060 if g == 0 else 0.105)
                for bi, b in enumerate(bs):
                    for c4 in range(2):
                        rows4 = slice(c4 * 512, (c4 + 1) * 512)
                        asl = slice(bi * NCHUNK + 4 * c4, bi * NCHUNK + 4 * c4 + 4)
                        v_t = vpool.tile([128, 4, L], F16)
                        nc.scalar.activation(
                            v_t[:], a_grp[:, asl, :],
                            mybir.ActivationFunctionType.Silu, scale=-1.0,
                        )
                        # ent = lg - v, in place over v.  POOL runs ~2x slow
                        # under SBUF-port contention, so alternate with DVE in
                        # the big group and keep the last group off POOL
                        # entirely (no slow tail).
                        if g == 1:
                            nc.vector.tensor_sub(v_t[:], lg_grp[:, asl, :], v_t[:])
                        else:
                            nc.gpsimd.tensor_sub(v_t[:], lg_grp[:, asl, :], v_t[:])
                        nc.sync.dma_start(
                            out=entropy_o[rows4, b, :].rearrange("(t p) l -> p t l", p=128),
                            in_=v_t[:],
                        )

    return nc


_NC = None
_THR = None


def _get_nc():
    global _NC
    if _NC is None:
        _NC = _build_nc()
    return _NC


def _get_thr():
    """t = logit(u) with u = the exact uniforms jax.random.bernoulli(key(42))
    draws inside the reference.  Input-independent => precomputed constant.
    fp16 threshold rounding flips ~300 of 33.5M samples (norm gate allows
    ~6700).  u == 0 gives t = -inf; clamp to -65000 so the fp16 diag
    (clamped to ~-65300) still compares below it."""
    global _THR
    if _THR is None:
        import jax
        cpu = jax.devices("cpu")[0]
        with jax.default_device(cpu):
            u = np.asarray(
                jax.random.uniform(
                    jax.random.key(42), (L, B, L), dtype=np.float32
                )
            )
        u64 = u.astype(np.float64)
        with np.errstate(divide="ignore"):
            t = np.log(u64) - np.log1p(-u64)
        t = np.clip(t, -65000.0, 65000.0)
        _THR = t.astype(np.float16)
    return _THR


def _shard_inputs(encoder_output, W, l):
    """Build the per-core input maps (also used by test.py)."""
    encoder_output = np.ascontiguousarray(encoder_output, dtype=np.float32)
    W = np.ascontiguousarray(W, dtype=np.float32)
    l = np.ascontiguousarray(l, dtype=np.float32)
    thr = _get_thr()
    in_maps = []
    for i in range(N_CORES):
        bs = slice(i * BPC, (i + 1) * BPC)
        shard = thr[:, bs, :]
        # [L, BPC, L] -> [BPC, L/512, 128, 4, L]: row l = s*512 + t*128 + p
        tiled = np.ascontiguousarray(
            shard.reshape(NCHUNK // 4, 4, 128, BPC, L)
            .transpose(3, 0, 2, 1, 4)
        )
        in_maps.append({
            "enc": np.ascontiguousarray(encoder_output[bs].transpose(0, 2, 1)),
            "w_in": np.ascontiguousarray(W.T),
            "lbias": l,
            "thr": tiled,
        })
    return in_maps


def _unshard(results):
    samples = np.concatenate(
        [np.asarray(r["samples_o"]).astype(np.float32) for r in results], axis=1)
    masked = np.concatenate(
        [np.asarray(r["masked_o"]).astype(np.float32) for r in results], axis=1)
    entropy = np.concatenate(
        [np.asarray(r["entropy_o"]).astype(np.float32) for r in results], axis=1)
    # fp16 clamps the diagonal (-1e8 -> -65300); true value is
    # -1e8 + logits_ii + l = -1e8 * (1 + O(6e-7)): write the constant.
    ar = np.arange(L)
    masked[ar, :, ar] = -NEG_BIG
    return samples, masked, entropy


def kernel(encoder_output, W, l):
    in_maps = _shard_inputs(encoder_output, W, l)
    nc = _get_nc()
    res = run_bass_kernel_spmd(nc, in_maps, core_ids=list(range(N_CORES)))
    return _unshard(res.results)
